# revision 52
# baseline (speedup 1.0000x reference)
"""
GeneNetworkGreensFunction kernel for 8 Trainium2 NeuronCores.

Math (Woodbury, real factorization): with z = omega + i*eta, a = 1/(z-d),
H = U U^T + diag(d), M = (I_r - U^T diag(a) U)^{-1} (complex symmetric):
    G = diag(a) + diag(a) U M U^T diag(a)
Two device paths produce OSCALE*|G|^2 in f8e4:

* std path (9 panels/strip): |G|^2 = (Lr R^T)^2 + (Li R^T)^2 with
  Lr/Li/R real [n,32] bf16 (two-band K=64 lhs + zero-band rhs trick);
  ACT squares ps_re to SBUF, a custom DVE op ships sq(ps_im)+sq_re.

* quadratic-form path (8 panels/strip, both strips share the features):
  |G_ij|^2 = W_i . F_j exactly, with W_i = s_i^2 vec(u_i u_i^T) and
  F_j = s_j^2 vec(yr_j yr_j^T + yi_j yi_j^T), y = M u (528 sym features).
  fp8e4 features feed three chained DoubleRow matmuls (k-planes
  128+128+8, K_eff 528) that land |G|^2 straight in PSUM, so each
  column needs ONE evacuation op (scaled copy, split ACT/DVE) instead
  of an ACT square + DVE sqadd. This is what breaks the two-touch
  PSUM-evacuation floor that bounds the std-only design.

The host decodes f8, divides by OSCALE, takes sqrt, patches the
diagonal in complex128, and mirrors the lower triangle. The Pool
engine (no PSUM port, otherwise idle) issues mid-stream output DMAs
via SWDGE so the drain overlaps compute; SP pre-waits on the tiny
terminal piece for the shortest last-compute -> done chain.
"""

import sys

for _p in ("/opt/trn_rl_repo",):
    if _p not in sys.path:
        sys.path.insert(0, _p)

import numpy as np

N = 4096
RANK = 32
CLAMP = 10.0
NCORES = 8
RH = 256                     # rows per block (row-block height)
MT = 128
PW = 256                     # panel width (columns)
NPANEL = 17                  # panels per core
OUTW = NPANEL * PW           # 4352 columns per 128-row strip
K2 = 2 * RANK
# consts columns: [lhs_m0 (lr|li, 256) | rhs 4352 | lhs_m1 (lr|li, 256)]
RB = 2 * MT                  # rhs base column
LB = RB + OUTW               # lhs base for strip m=1
CW = LB + 2 * MT
OSCALE = 2048.0              # device ships OSCALE*|G|^2 in f8e4
NQF = 8                      # panels per strip on the QF path (slots 5..12)
QF0 = 5                      # first QF panel slot
QCOLS = NQF * PW             # 2048 QF columns per strip
NFEAT = 528                  # rank*(rank+1)/2 symmetric features
WA = 640                     # QF evac columns per 1024 going to ACT (rest DVE)

_CACHE = {}

def _get_sqadd():
    """Register (once) and return the fused square-accumulate custom DVE op:
    out = sq(Src0) + Src1. Src0 is the single PSUM operand the hardware
    allows per instruction; Src1 comes from SBUF."""
    import concourse.dve_ops as dve_ops

    for op in dve_ops.OPS:
        if op.name == "SQADD_ANT":
            return op

    from concourse.dve_spec import Spec, Src0, Src1, sq, lower, _has_src1
    from concourse.dve_uop import DveOpSpec

    spec = Spec(
        body=sq(Src0) + Src1,
        reference=lambda in0, in1, s0, s1, imm2: (
            in0.astype(np.float32) ** 2 + in1.astype(np.float32)
        ),
    )
    opcode = max(dve_ops._SUB_OPCODE_FOR_NAME.values()) + 1
    assert opcode < 0x20, "no free custom-DVE opcode row"
    shas = {}
    for ver in ("v3", "v4"):
        try:
            uops = lower(spec, ver=ver)
            shas[ver] = DveOpSpec(
                name="SQADD_ANT", opcode=opcode, uops=uops, rd1_en=_has_src1(spec)
            ).sha(ver)
        except Exception:
            pass
    assert shas, "SQADD spec failed to lower for all DVE vers"
    op = dve_ops.DveOp("SQADD_ANT", spec, subdim=False, uops_sha=shas)
    dve_ops.OPS.append(op)
    dve_ops.CUSTOM_DVE_SPECS["SQADD_ANT"] = spec
    dve_ops._SUB_OPCODE_FOR_NAME["SQADD_ANT"] = opcode
    return op


def _woodbury_host(omega, U, d, log_eta):
    """complex128 host algebra. Returns (Lr, Li, R [n,32] f64, UM, a, diag).
    Lr/Li/R carry OSCALE**0.25 so (LrR)^2+(LiR)^2 = OSCALE*|G|^2."""
    U = np.asarray(U, np.float64)
    d = np.asarray(d, np.float64)
    eta = float(np.exp(np.float64(np.asarray(log_eta))))
    z = complex(float(np.asarray(omega)), eta)
    a = 1.0 / (z - d)                          # [n] complex128
    B = U.T @ (a[:, None] * U)                 # [r, r] complex
    M = np.linalg.inv(np.eye(RANK) - B)        # [r, r] complex symmetric
    UM = U @ M                                 # [n, r] complex
    s = np.abs(a)
    c = OSCALE ** 0.25
    Lr = (c * s)[:, None] * UM.real
    Li = (c * s)[:, None] * UM.imag
    R = (c * s)[:, None] * U
    diag = a + a * a * np.einsum("ij,ij->i", UM, U)  # G_ii
    return Lr, Li, R, U, UM, np.abs(a) ** 2, diag


def _qf_features(U, UM, s2):
    """528-dim symmetric features: W_i.F_j = OSCALE*|G_ij|^2 (i != j).
    Returns (Wq [n,528] f8-ready scaled, Pq, inv_scale)."""
    iu0, iu1 = np.triu_indices(RANK)
    w = np.where(iu0 == iu1, 1.0, 2.0)
    yr = UM.real
    yi = UM.imag
    c = OSCALE ** 0.5
    Wf = (c * s2)[:, None] * (U[:, iu0] * U[:, iu1]) * w
    Pf = (c * s2)[:, None] * (yr[:, iu0] * yr[:, iu1] + yi[:, iu0] * yi[:, iu1])
    sa = 128.0 / max(np.abs(Wf).max(), 1e-30)
    sb = 128.0 / max(np.abs(Pf).max(), 1e-30)
    return Wf * sa, Pf * sb, 1.0 / (sa * sb)


def _panel_map():
    """panels[core] = 17 (rb, j): rb = 256-row block, j = 256-col tile,
    upper triangle j >= rb; core r owns row-blocks r and 15-r completely.
    Slot order = consts column order = compute order: [diag_a, remnant,
    diag_b, other, other, 8x QF (all band-0: row-block r), 4x other].
    Slots 5..12 must be band-0 strictly-upper panels on EVERY core so the
    shared program can use one compile-time lhsT layout (core 7 has
    exactly 8 such panels)."""
    out = []
    for r in range(8):
        seq = [(r, j) for j in range(r, 16)] + [
            (15 - r, j) for j in range(15 - r, 16)
        ]
        assert len(seq) == NPANEL
        diag_a, diag_b = (r, r), (15 - r, 15 - r)
        # remnant must be a strictly-upper panel (its m1 half is NOT trimmed);
        # for r=0 the natural pick (15,15) collides with diag_b
        remnant = (15 - r, 15) if r != 0 else (0, 15)
        rest = [p for p in seq if p not in (diag_a, diag_b, remnant)]
        b0 = [p for p in rest if p[0] == r]
        b1 = [p for p in rest if p[0] != r]
        assert len(b0) >= NQF, (r, len(b0))
        qf = b0[:NQF]
        others = b0[NQF:] + b1
        assert len(others) == NPANEL - 3 - NQF
        panels = [diag_a, remnant, diag_b] + others[:2] + qf + others[2:]
        assert len(panels) == NPANEL and len(set(panels)) == NPANEL
        out.append(panels)
    return out


PANELS = _panel_map()

# global chunk order (strip m, p0, p1, kind), arranged so each chunk's
# emission position matches its input-DMA arrival time (engine queues
# are in-order, so a stalled head blocks everything behind it) and so
# QF evacuations interleave with std work instead of piling up at the
# end. The tiny remnant closes so the terminal piece has the shortest
# last-compute -> done chain.
CHUNKS = [
    (0, 0, 1, "s"), (0, 1, 3, "s"), (0, 3, 5, "s"),
    (0, 5, 9, "q"), (1, 0, 5, "s"), (1, 5, 9, "q"),
    (0, 13, 17, "s"), (1, 1, 2, "s"), (1, 13, 17, "s"),
    (0, 9, 13, "q"), (1, 9, 13, "q"),
]
# output pieces issued right after the keyed chunk: (engine, col0, width).
# The closing QF pair alternates PSUM pools so its matmuls don't wait on
# each other's evacuation, and the final piece is a small pre-waited one.
PIECES = {
    (0, 3, 5): ("pool", 0, 1280), (0, 13, 17): ("sp", 3328, 1024),
    (1, 13, 17): ("pool", 2816, 1024), (1, 1, 2): ("sp", 3840, 256),
    (1, 5, 9): ("pool", 0, 1792), (0, 5, 9): ("pool", 1280, 1024),
    (0, 9, 13): ("pool", 2304, 1024), (1, 9, 13): ("sp", 1792, 1024),
}


def _m1_out_col(t):
    """o-tile column offset for panel slot t in strip m1 (diag panels keep
    only their right 128 cols; the remnant slot goes last)."""
    if t == 0:
        return 0
    if t == 2:
        return 128
    if t == 1:
        return 256 + 14 * PW
    return 256 + (t - 3) * PW


def _build_program():
    import concourse.bass as bass
    import concourse.mybir as mybir
    import concourse.tile as tile
    from concourse import bacc
    from concourse.bass import ds, ts

    f32 = mybir.dt.float32
    bf16 = mybir.dt.bfloat16
    f8 = mybir.dt.float8e4
    DR = mybir.MatmulPerfMode.DoubleRow
    COPY = mybir.ActivationFunctionType.Copy
    sqadd = _get_sqadd()

    nc = bacc.Bacc(
        "TRN2", target_bir_lowering=False, debug=False, num_devices=NCORES
    )

    consts = nc.declare_dram_parameter("consts", [K2, CW], bf16, isOutput=False)
    # feature tensors are packed chunk-major: [chunk, plane, 1024 cols]
    qf_phi1 = nc.declare_dram_parameter("qf_phi1", [MT, 4 * QCOLS], f8, isOutput=False)
    qf_phi3 = nc.declare_dram_parameter("qf_phi3", [8, 2 * QCOLS], f8, isOutput=False)
    qf_wm = nc.declare_dram_parameter("qf_wm", [MT, 6 * 2 * MT], f8, isOutput=False)
    qf_scal = nc.declare_dram_parameter("qf_scal", [MT, 1], f32, isOutput=False)
    out = nc.declare_dram_parameter("out", [MT, 2 * OUTW], f8, isOutput=True)

    with tile.TileContext(nc) as tc:
        with (
            tc.tile_pool(name="consts", bufs=1) as cpool,
            tc.tile_pool(name="qfc", bufs=7) as qpool,
            tc.tile_pool(name="psre", bufs=2, space="PSUM") as pr_pool,
            tc.tile_pool(name="psim", bufs=2, space="PSUM") as pi_pool,
            tc.tile_pool(name="ssum", bufs=3) as spool,
            tc.tile_pool(name="outp", bufs=2) as opool,
        ):
            # std consts split into one tile per input DMA: disjoint tiles
            # keep the DMA stream free of tile-granular WAW serialization
            t_ca = cpool.tile([K2, RB + 256], bf16, tag="ca", name="t_ca")
            t_cb = cpool.tile([K2, 1024], bf16, tag="cb", name="t_cb")
            t_cc = cpool.tile([K2, 4 * PW], bf16, tag="cc", name="t_cc")
            t_cd = cpool.tile([K2, 2 * MT], bf16, tag="cd", name="t_cd")
            t_phi1 = [qpool.tile([MT, 4, 1024], f8, tag=f"phi1{i}", name=f"t_phi1{i}") for i in range(2)]
            t_phi3 = [qpool.tile([8, 2, 1024], f8, tag=f"phi3{i}", name=f"t_phi3{i}") for i in range(2)]
            t_wm = qpool.tile([MT, 6, 2 * MT], f8, tag="wm")
            t_scal = qpool.tile([MT, 1], f32, tag="scal")
            # PE p-state warm-up: tiny matmuls on zeroed scratch, finishing
            # before the first input DMA lands. pe ramp time accrues from the
            # first matmul, so the real stream runs at full clock ~1us sooner.
            warm = spool.tile([K2, MT + 96], bf16, tag="sqre")
            nc.vector.memset(warm[:], 0)
            ps_w = pr_pool.tile([MT, 1024], f32, tag="psre")
            for _ in range(8):
                nc.tensor.matmul(
                    ps_w[:, ds(0, 96)],
                    warm[:, ds(0, MT)],
                    warm[:, ds(MT, 96)],
                    start=True, stop=True,
                )
            # input DMAs in consumption order. SP/HWDGE: std stream (incl.
            # the tiny lhs_m1 early so both strips' std chunks can run
            # during the ramp), then the per-chunk feature halves.
            # Pool/SWDGE: the small QF tensors (keeps SP's HWDGE line
            # clear; Pool is idle early).
            # the v1 cost model serializes DMA transfer busy per ISSUING
            # queue (busy = per-partition-bytes * 0.385ns; 500ns floor per
            # DMA) while the SP and Pool queues transfer concurrently. The
            # two streams below are each ordered by first consumption and
            # balanced so every tensor lands just ahead of its chunk.
            nc.sync.dma_start(
                out=t_ca[:], in_=consts[:, ds(0, RB + 256)]
            )                                               # lhs_m0 + slot 0
            nc.sync.dma_start(
                out=t_cb[:], in_=consts[:, ds(RB + 256, 1024)]
            )                                               # slots 1-4
            for i in range(2):
                # split by k-planes (the flat layout is plane-major): the
                # first half feeds the chunk's mm1s ~0.8us before mm2's
                # planes land, letting the PE start accumulation chains early
                for h in range(2):
                    nc.sync.dma_start(
                        out=t_phi1[i][:, ds(2 * h, 2), 0:1024],
                        in_=qf_phi1[:, ds(i * 4096 + h * 2048, 2048)],
                    )
            nc.gpsimd.dma_start(
                out=t_cd[:], in_=consts[:, ds(LB, CW - LB)]
            )                                               # lhs_m1
            nc.gpsimd.dma_start(out=t_wm[:], in_=qf_wm[:])
            nc.gpsimd.dma_start(out=t_phi3[0][:], in_=qf_phi3[:, ds(0, 2048)])
            nc.gpsimd.dma_start(out=t_scal[:], in_=qf_scal[:])
            nc.gpsimd.dma_start(
                out=t_cc[:], in_=consts[:, ds(RB + 13 * PW, 4 * PW)]
            )                                               # slots 13-16
            nc.gpsimd.dma_start(out=t_phi3[1][:], in_=qf_phi3[:, ds(2048, 2048)])


            def slot_rhs(t, off=0, wd=PW):
                """std rhs AP for panel slot t (+off, width wd)."""
                if t == 0:
                    return t_ca[:, ds(RB + off, wd)]
                if t <= 4:
                    return t_cb[:, ds((t - 1) * PW + off, wd)]
                return t_cc[:, ds((t - 13) * PW + off, wd)]

            o_m = [opool.tile([MT, OUTW], f8, tag="o", name=f"o{m}")
                   for m in range(2)]
            for m, p0, p1, kind in CHUNKS:
                lhs_t = t_ca if m == 0 else t_cd
                lr = lhs_t[:, ds(0, MT)]
                li = lhs_t[:, ds(MT, MT)]
                o = o_m[m]
                if True:
                    if kind == "q":
                        # quadratic-form chunk: 3 chained DoubleRow matmuls
                        # per 256-col tile put OSCALE*|G|^2 in PSUM directly
                        ch = (p0 - QF0) // 4       # feature chunk index
                        w = (p1 - p0) * PW
                        ob = _m1_out_col(p0) if m == 1 else p0 * PW
                        phi1c, phi3c = t_phi1[ch], t_phi3[ch]
                        qpool_ps = pr_pool if m == 0 else pi_pool
                        psq = qpool_ps.tile(
                            [MT, 1024], f32,
                            tag="psre" if m == 0 else "psim", name="psq",
                        )
                        for t in range(w // PW):
                            po = psq[:, ds(t * PW, PW)]
                            rc = t * PW
                            nc.tensor.matmul(
                                po, t_wm[:, 0:2, ts(m, MT)],
                                phi1c[:, 0:2, ds(rc, PW)],
                                start=True, stop=False, perf_mode=DR,
                            )
                            nc.tensor.matmul(
                                po, t_wm[:, 2:4, ts(m, MT)],
                                phi1c[:, 2:4, ds(rc, PW)],
                                start=False, stop=False, perf_mode=DR,
                            )
                            nc.tensor.matmul(
                                po, t_wm[0:8, 4:6, ts(m, MT)],
                                phi3c[0:8, 0:2, ds(rc, PW)],
                                start=False, stop=True, perf_mode=DR,
                            )
                        # single evacuation per column, split ACT/DVE; the
                        # closing pair leans on ACT (DVE still drains the
                        # last std sqadd when these become ready)
                        wa = (832 if m == 1 else 680) if p0 == 9 else WA
                        nc.scalar.activation(
                            o[:, ds(ob, wa)], psq[:, ds(0, wa)],
                            COPY, scale=t_scal[:, 0:1],
                        )
                        nc.vector.tensor_scalar_mul(
                            o[:, ds(ob + wa, w - wa)], psq[:, ds(wa, w - wa)],
                            t_scal[:, 0:1],
                        )
                    else:
                        if m == 1 and p0 == 0:
                            # head chunk incl. trimmed diag panels (right 128s)
                            w = 768
                            segs = (
                                (0, slot_rhs(0, 128, 128)),
                                (128, slot_rhs(2, 128, 128)),
                                (256, slot_rhs(3)),
                                (512, slot_rhs(4)),
                            )
                            ob = 0
                        elif m == 1 and p0 == 1:
                            # terminal remnant piece
                            w = 256
                            segs = ((0, slot_rhs(1)),)
                            ob = 256 + 14 * PW
                        else:
                            w = (p1 - p0) * PW
                            segs = tuple(
                                ((t - p0) * PW, slot_rhs(t))
                                for t in range(p0, p1)
                            )
                            ob = _m1_out_col(p0) if m == 1 else p0 * PW
                        ps_re = pr_pool.tile([MT, 1024], f32, tag="psre")
                        ps_im = pi_pool.tile([MT, 1024], f32, tag="psim")
                        for lhs, ps in ((lr, ps_re), (li, ps_im)):
                            for d0, rr in segs:
                                sw_ = rr.shape[-1]
                                nc.tensor.matmul(
                                    ps[:, ds(d0, sw_)], lhs, rr,
                                    start=True, stop=True,
                                )
                        sq_re = spool.tile([MT, 1024], f32, tag="sqre")
                        nc.scalar.square(
                            sq_re[:, ds(0, w)], ps_re[:, ds(0, w)]
                        )
                        nc.vector._custom_dve(
                            sqadd,
                            out=o[:, ds(ob, w)],
                            in0=ps_im[:, ds(0, w)],
                            in1=sq_re[:, ds(0, w)],
                        )
                    piece = PIECES.get((m, p0, p1))
                    if piece is not None:
                        # mid-stream pieces issue from the Pool engine
                        # (SWDGE keeps SP's HWDGE line clear); the terminal
                        # piece from SP, whose queue is empty and pre-waiting
                        # by then (post-wait latency 625 HWDGE + 650 DGE +
                        # xfer + 900 sem).
                        eng, c0, pw_ = piece
                        (nc.gpsimd if eng == "pool" else nc.sync).dma_start(
                            out=out[:, ds(m * OUTW + c0, pw_)],
                            in_=o[:, ds(c0, pw_)],
                        )
    nc.finalize()
    return nc


def _prepare(omega, H_low_rank, H_diag, log_eta):
    import ml_dtypes

    Lr, Li, R, U, UM, s2, diag = _woodbury_host(omega, H_low_rank, H_diag, log_eta)
    Wsc, Psc, inv_scale = _qf_features(U, UM, s2)
    bf16 = ml_dtypes.bfloat16
    f8 = ml_dtypes.float8_e4m3
    LrT = np.ascontiguousarray(Lr.T).astype(bf16)   # [32, n]
    LiT = np.ascontiguousarray(Li.T).astype(bf16)
    RT = np.ascontiguousarray(R.T).astype(bf16)
    WqT = np.ascontiguousarray(Wsc.T).astype(f8)    # [528, n] rows-features
    PqT = np.ascontiguousarray(Psc.T).astype(f8)    # [528, n] cols-features
    scal = np.full((MT, 1), inv_scale, np.float32)

    in_maps = []
    for c in range(NCORES):
        blocks = [c, 15 - c]
        consts = np.zeros((K2, CW), bf16)
        for band, rb in enumerate(blocks):
            sl = slice(band * RANK, (band + 1) * RANK)
            for m in range(2):
                lb = 0 if m == 0 else LB
                rs = slice(rb * RH + m * MT, rb * RH + (m + 1) * MT)
                consts[sl, lb : lb + MT] = LrT[:, rs]
                consts[sl, lb + MT : lb + 2 * MT] = LiT[:, rs]
        for t, (rb, j) in enumerate(PANELS[c]):
            if QF0 <= t < QF0 + NQF:
                continue                       # QF panels: no std rhs
            band = blocks.index(rb)
            sl = slice(band * RANK, (band + 1) * RANK)
            consts[sl, RB + t * PW : RB + (t + 1) * PW] = RT[
                :, j * PW : (j + 1) * PW
            ]
        # QF features: panels at slots QF0..QF0+NQF-1, all band 0 (block c),
        # packed chunk-major ([chunk, plane, 1024]) so each chunk's features
        # arrive in one contiguous DMA just ahead of its matmuls
        phi1 = np.zeros((MT, 2, 4, 1024), f8)
        phi3 = np.zeros((8, 2, 2, 1024), f8)
        for ch in range(2):
            qcols = np.concatenate([
                np.arange(j * PW, (j + 1) * PW)
                for (rb, j) in PANELS[c][QF0 + 4 * ch : QF0 + 4 * (ch + 1)]
            ])
            phi1[:, ch] = PqT[:512, qcols].reshape(4, MT, 1024).transpose(1, 0, 2)
            phi3[:, ch] = PqT[512:, qcols].reshape(2, 8, 1024).transpose(1, 0, 2)
        wm = np.zeros((MT, 6, 2 * MT), f8)
        for m in range(2):
            rows = np.arange(c * RH + m * MT, c * RH + (m + 1) * MT)
            wm[:, 0:4, m * MT : (m + 1) * MT] = (
                WqT[:512, rows].reshape(4, MT, MT).transpose(1, 0, 2)
            )
            wm[0:8, 4:6, m * MT : (m + 1) * MT] = (
                WqT[512:, rows].reshape(2, 8, MT).transpose(1, 0, 2)
            )
        in_maps.append({
            "consts": consts,
            "qf_phi1": np.ascontiguousarray(phi1).reshape(MT, 4 * QCOLS),
            "qf_phi3": np.ascontiguousarray(phi3).reshape(8, 2 * QCOLS),
            "qf_wm": np.ascontiguousarray(wm).reshape(MT, 6 * 2 * MT),
            "qf_scal": scal,
        })
    return in_maps, np.minimum(np.abs(diag), CLAMP).astype(np.float32)


def _assemble(results, diag_vals):
    full = np.empty((N, N), np.float32)
    for c in range(NCORES):
        o = results[c]["out"]                      # [128, 8704] f8e4
        o = np.asarray(o).astype(np.float32) * (1.0 / OSCALE)
        for t, (rb, j) in enumerate(PANELS[c]):
            full[
                rb * RH : rb * RH + MT, j * PW : (j + 1) * PW
            ] = o[:, t * PW : (t + 1) * PW]
            if t in (0, 2):
                c1 = OUTW + _m1_out_col(t)
                full[
                    rb * RH + MT : rb * RH + 2 * MT,
                    j * PW + 128 : (j + 1) * PW,
                ] = o[:, c1 : c1 + 128]
            else:
                c1 = OUTW + _m1_out_col(t)
                full[
                    rb * RH + MT : rb * RH + 2 * MT,
                    j * PW : (j + 1) * PW,
                ] = o[:, c1 : c1 + PW]
        for t in (0, 2):
            rb, j = PANELS[c][t]
            b = rb * RH
            full[b + MT : b + 2 * MT, b : b + MT] = full[
                b : b + MT, b + MT : b + 2 * MT
            ].T
    # mirror strictly-upper blocks
    for c in range(NCORES):
        for rb, j in PANELS[c]:
            if j > rb:
                full[j * PW : (j + 1) * PW, rb * RH : (rb + 1) * RH] = full[
                    rb * RH : (rb + 1) * RH, j * PW : (j + 1) * PW
                ].T
    out = np.sqrt(np.maximum(full, 0.0))
    np.fill_diagonal(out, diag_vals)
    return out


def kernel(omega, H_low_rank, H_diag, log_eta):
    from concourse.bass_utils import run_bass_kernel_spmd

    in_maps, diag_vals = _prepare(omega, H_low_rank, H_diag, log_eta)
    if "nc" not in _CACHE:
        _CACHE["nc"] = _build_program()
    res = run_bass_kernel_spmd(_CACHE["nc"], in_maps, list(range(NCORES)))
    return _assemble(res.results, diag_vals)
